# revision 5
# baseline (speedup 1.0000x reference)
"""Bass/Tile kernel for nn_CausalSelfAttention (GQA + RMS-norm + RoPE + sliding window).

Sharding: tensor-parallel over heads (2 q-heads + 1 kv-head per core, kv
replicated x2), full 4096-seq per core. Inputs arrive as 1/8-sized slices;
the full x^T (+ rope tables) is assembled on device with an AllGather over
NeuronLink, and the out-projection partials are summed with a ReduceScatter,
so each core returns a disjoint [256, 4096] slice of out^T in bf16.

Device layouts are transpose-free:
  - host passes x^T seq-slice and W^T slices
  - projections produce q^T/k^T [hd, seq] (lhsT = W tile) and v [seq, hd]
    (lhsT = x^T tile) directly
  - scores^T [sk, sq] = k_tile^T.T @ q^T ; PV: y^T += v_tile.T @ probs^T
  - out^T partial = Wo_slice^T.T @ y^T ; ReduceScatter(add) -> own 256 rows

Window/causal masking: per 512-query block b, only k-tiles
[max(0, 4b-8) .. 4b+3] are computed (compile-time clamp at the sequence
start), with affine_select zeroing the triangular window/causal edges
post-exp. RMS-norm scales fold into the RoPE multiply; 1/sqrt(hd) folds
into the q-side scale; softmax needs no running max (rms-normed logits are
bounded by sqrt(128)).
"""

import sys

if "/opt/trn_rl_repo" not in sys.path:
    sys.path.insert(0, "/opt/trn_rl_repo")

import ml_dtypes
import numpy as np

import concourse.bass as bass
import concourse.mybir as mybir
import concourse.tile as tile
from concourse import bacc, bass_isa, bass_utils

f32 = mybir.dt.float32
f32r = mybir.dt.float32r
bf16 = mybir.dt.bfloat16
AF = mybir.ActivationFunctionType

D = 2048
S = 4096
NH = 16
NKV = 4
HD = 128
NCORE = 8
H = NH // NCORE             # 2 q-heads per core
OW = H * HD                 # 256 out dims per core
SB = 512                    # seq block
NB = 512                    # matmul moving block
NBLK = S // SB              # 8
NDT = D // 128              # 16
GROWS = D + 128             # x rows + 64 cos rows + 64 sin rows per gather block
EPS = 1.1920929e-07


def build_program():
    nc = bacc.Bacc(
        "TRN2",
        target_bir_lowering=False,
        debug=False,
        enable_asserts=False,
        num_devices=8,
    )
    gin = nc.dram_tensor("gin", [GROWS, SB], bf16, kind="ExternalInput").ap()
    wqd = nc.dram_tensor("wqd", [D, OW], bf16, kind="ExternalInput").ap()
    # half of the kv projection: Wk^T slice on even cores, Wv^T on odd; the
    # pair AllGather reassembles [Wk^T; Wv^T] on both cores of each kv pair
    wkvh = nc.dram_tensor("wkvh", [D, HD], bf16, kind="ExternalInput").ap()
    wod = nc.dram_tensor("wod", [OW, D], bf16, kind="ExternalInput").ap()
    outp = nc.dram_tensor("outp", [OW, S], bf16, kind="ExternalOutput").ap()

    with tile.TileContext(nc) as tc:
        with (
            tc.tile_pool(name="dram", bufs=1, space="DRAM") as dpool,
            tc.tile_pool(name="persist", bufs=1) as persist,
            tc.tile_pool(name="scratch", bufs=6) as sc,
            tc.tile_pool(name="rows", bufs=3) as rows,
        ):
            gbounce = dpool.tile([GROWS, SB], bf16)
            gall = dpool.tile([NCORE * GROWS, SB], bf16, addr_space="Shared")
            obounce = dpool.tile([D, S], bf16)
            oslice = dpool.tile([OW, S], bf16)

            wkvb = dpool.tile([D, HD], bf16)
            wkvp = dpool.tile([2 * D, HD], bf16)

            nc.gpsimd.dma_start(gbounce[:], gin)
            nc.gpsimd.collective_compute(
                "AllGather",
                mybir.AluOpType.bypass,
                replica_groups=[list(range(NCORE))],
                ins=[gbounce.opt()],
                outs=[gall.opt()],
            )
            nc.gpsimd.dma_start(wkvb[:], wkvh)
            nc.gpsimd.collective_compute(
                "AllGather",
                mybir.AluOpType.bypass,
                replica_groups=[[2 * p, 2 * p + 1] for p in range(NCORE // 2)],
                ins=[wkvb.opt()],
                outs=[wkvp.opt()],
            )

            # --- constants ---
            ones_col = persist.tile([128, 1], bf16)
            nc.vector.memset(ones_col, 1.0)
            eps_q = persist.tile([128, 1], f32)
            nc.vector.memset(eps_q, 128.0 * EPS)
            eps_k = persist.tile([128, 1], f32)
            nc.vector.memset(eps_k, EPS)

            # --- persistent weights / tables ---
            wq_sb = persist.tile([128, NDT, OW], bf16)
            nc.sync.dma_start(out=wq_sb, in_=wqd.rearrange("(c p) w -> p c w", p=128))
            wk_sb = persist.tile([128, NDT, HD], bf16)
            nc.sync.dma_start(
                out=wk_sb, in_=wkvp[0:D, :].rearrange("(c p) w -> p c w", p=128)
            )
            wv_sb = persist.tile([128, NDT, HD], bf16)
            nc.sync.dma_start(
                out=wv_sb, in_=wkvp[D : 2 * D, :].rearrange("(c p) w -> p c w", p=128)
            )
            wo_sb = persist.tile([128, H, D], bf16)
            nc.sync.dma_start(out=wo_sb, in_=wod.rearrange("(y p) d -> p y d", p=128))

            c2 = persist.tile([128, S], bf16)
            s2 = persist.tile([128, S], bf16)
            for b in range(NBLK):
                for half in range(2):
                    nc.sync.dma_start(
                        out=c2[64 * half : 64 * (half + 1), SB * b : SB * (b + 1)],
                        in_=gall[GROWS * b + D : GROWS * b + D + 64, :],
                    )
                    nc.sync.dma_start(
                        out=s2[64 * half : 64 * (half + 1), SB * b : SB * (b + 1)],
                        in_=gall[GROWS * b + D + 64 : GROWS * b + D + 128, :],
                    )

            qrot = persist.tile([128, H, S], bf16)
            krot = persist.tile([128, S], bf16)
            v_sb = persist.tile([128, S // 128, HD], bf16)
            yt = persist.tile([128, H, S], bf16)

            def drain_norm_rope(acc, out_slice, ctab, stab, s_scale, s_bias, nm):
                """acc: PSUM [128, NB] raw projection. Writes the rms-normed,
                rope-rotated (and, for q, 1/sqrt(hd)-scaled) result."""
                raw = sc.tile([128, NB], bf16, tag="big0", name=f"raw{nm}")
                nc.scalar.copy(out=raw, in_=acc)
                sqd_t = sc.tile([128, NB], bf16, tag="big1", name=f"sqd{nm}")
                nc.vector.tensor_mul(out=sqd_t, in0=raw, in1=raw)
                allr = sc.tile([128, NB], f32, tag="big2", name=f"allr{nm}")
                nc.gpsimd.partition_all_reduce(
                    allr, sqd_t, channels=128, reduce_op=bass_isa.ReduceOp.add
                )
                s_full = sc.tile([128, NB], f32, tag="big3", name=f"sf{nm}")
                nc.scalar.activation(
                    out=s_full, in_=allr, func=AF.Sqrt, bias=s_bias, scale=s_scale
                )
                a_full = sc.tile([128, NB], bf16, tag="big4", name=f"af{nm}")
                with nc.allow_low_precision(reason="f32r is 4-byte fp32 storage"):
                    nc.vector.reciprocal(out=a_full, in_=s_full)
                ca = sc.tile([128, NB], bf16, tag="big5", name=f"ca{nm}")
                nc.vector.tensor_mul(out=ca, in0=ctab, in1=a_full)
                sa = sc.tile([128, NB], bf16, tag="big6", name=f"sa{nm}")
                nc.vector.tensor_mul(out=sa, in0=stab, in1=a_full)
                t1 = sc.tile([128, NB], bf16, tag="big1", name=f"t1{nm}")
                nc.vector.tensor_mul(out=t1, in0=raw, in1=ca)
                t2 = sc.tile([128, NB], bf16, tag="big3", name=f"t2{nm}")
                nc.vector.tensor_mul(out=t2, in0=raw, in1=sa)
                usw = sc.tile([128, NB], bf16, tag="big2", name=f"usw{nm}")
                nc.gpsimd.tensor_copy(out=usw[0:64, :], in_=t2[64:128, :])
                nc.gpsimd.tensor_copy(out=usw[64:128, :], in_=t2[0:64, :])
                nc.vector.tensor_add(
                    out=out_slice[0:64, :], in0=t1[0:64, :], in1=usw[0:64, :]
                )
                nc.vector.tensor_sub(
                    out=out_slice[64:128, :], in0=t1[64:128, :], in1=usw[64:128, :]
                )

            # ============ Phase P: q/k/v projections over all seq blocks ============
            with tc.tile_pool(name="psP", bufs=8, space="PSUM") as psP:
                with tc.tile_pool(name="xstream", bufs=4) as xs:
                    for b in range(NBLK):
                        qacc = [
                            psP.tile([128, NB], f32, tag="acc", name=f"qacc{b}_{h}")
                            for h in range(H)
                        ]
                        kacc = psP.tile([128, NB], f32, tag="acc", name=f"kacc{b}")
                        vacc = [
                            psP.tile([128, HD], f32, tag="acc", name=f"vacc{b}_{lt}")
                            for lt in range(4)
                        ]
                        for dc in range(NDT // 8):
                            xt8 = xs.tile(
                                [128, 8, NB], bf16, tag="xt", bufs=3, name=f"xt{b}_{dc}"
                            )
                            nc.sync.dma_start(
                                out=xt8,
                                in_=gall[
                                    GROWS * b + 1024 * dc : GROWS * b
                                    + 1024 * (dc + 1),
                                    :,
                                ].rearrange("(c p) w -> p c w", p=128),
                            )
                            for dl in range(8):
                                d = 8 * dc + dl
                                for h in range(H):
                                    nc.tensor.matmul(
                                        qacc[h],
                                        lhsT=wq_sb[:, d, HD * h : HD * (h + 1)],
                                        rhs=xt8[:, dl, :],
                                        start=(d == 0),
                                        stop=(d == NDT - 1),
                                    )
                                nc.tensor.matmul(
                                    kacc,
                                    lhsT=wk_sb[:, d, :],
                                    rhs=xt8[:, dl, :],
                                    start=(d == 0),
                                    stop=(d == NDT - 1),
                                )
                                for lt in range(4):
                                    nc.tensor.matmul(
                                        vacc[lt],
                                        lhsT=xt8[:, dl, 128 * lt : 128 * (lt + 1)],
                                        rhs=wv_sb[:, d, :],
                                        start=(d == 0),
                                        stop=(d == NDT - 1),
                                    )
                        for h in range(H):
                            drain_norm_rope(
                                qacc[h],
                                qrot[:, h, SB * b : SB * (b + 1)],
                                c2[:, SB * b : SB * (b + 1)],
                                s2[:, SB * b : SB * (b + 1)],
                                1.0,
                                eps_q,
                                f"q{b}_{h}",
                            )
                        drain_norm_rope(
                            kacc,
                            krot[:, SB * b : SB * (b + 1)],
                            c2[:, SB * b : SB * (b + 1)],
                            s2[:, SB * b : SB * (b + 1)],
                            1.0 / 128.0,
                            eps_k,
                            f"k{b}",
                        )
                        for lt in range(4):
                            nc.scalar.copy(
                                out=v_sb[:, 4 * b + lt, :], in_=vacc[lt]
                            )

            # ============ Phase A: attention ============
            with tc.tile_pool(name="probs", bufs=6) as pp, tc.tile_pool(
                name="psA", bufs=4, space="PSUM"
            ) as psA:
                for b in range(NBLK):
                    lo_i = 8 - min(8, 4 * b)  # first valid relative k-tile
                    for h in range(H):
                        nm = f"a{b}_{h}"
                        yacc = psA.tile([128, NB], f32, tag="y", name=f"yacc{nm}")
                        racc = psA.tile([1, NB], f32, tag="y", name=f"racc{nm}")
                        for ip in range(lo_i // 2, 6):
                            i0 = 2 * ip
                            sacc = psA.tile(
                                [128, 2, NB], f32, tag="s2", bufs=2,
                                name=f"sacc{nm}_{ip}",
                            )
                            for jj in range(2):
                                i = i0 + jj
                                t = 4 * b - 8 + i  # absolute k-tile
                                nc.tensor.matmul(
                                    sacc[:, jj, :],
                                    lhsT=krot[:, 128 * t : 128 * (t + 1)],
                                    rhs=qrot[:, h, SB * b : SB * (b + 1)],
                                    start=True,
                                    stop=True,
                                )
                            pt = pp.tile(
                                [128, 2, NB], bf16, tag="pt", name=f"pt{nm}_{ip}"
                            )
                            nc.scalar.activation(
                                out=pt, in_=sacc, func=AF.Exp, bias=0.0, scale=1.0
                            )
                            if i0 < 4:
                                # window edge: keep f - p <= 128*(i0+jj) - 1
                                nc.gpsimd.affine_select(
                                    out=pt,
                                    in_=pt,
                                    pattern=[[128, 2], [-1, NB]],
                                    compare_op=mybir.AluOpType.is_ge,
                                    fill=0.0,
                                    base=128 * i0 - 1,
                                    channel_multiplier=1,
                                )
                            elif i0 >= 8:
                                # causal edge: keep f - p >= 128*(i0+jj-8)
                                nc.gpsimd.affine_select(
                                    out=pt,
                                    in_=pt,
                                    pattern=[[-128, 2], [1, NB]],
                                    compare_op=mybir.AluOpType.is_ge,
                                    fill=0.0,
                                    base=-128 * (i0 - 8),
                                    channel_multiplier=-1,
                                )
                            for jj in range(2):
                                i = i0 + jj
                                t = 4 * b - 8 + i
                                nc.tensor.matmul(
                                    yacc,
                                    lhsT=v_sb[:, t, :],
                                    rhs=pt[:, jj, :],
                                    start=(i == lo_i),
                                    stop=(i == 11),
                                )
                                nc.tensor.matmul(
                                    racc,
                                    lhsT=ones_col,
                                    rhs=pt[:, jj, :],
                                    start=(i == lo_i),
                                    stop=(i == 11),
                                )
                        rinv = rows.tile([1, NB], f32r, tag="r1", name=f"rinv{nm}")
                        with nc.allow_low_precision(reason="f32r 4-byte"):
                            nc.vector.reciprocal(out=rinv, in_=racc)
                        rb = sc.tile([128, NB], f32r, tag="big5", name=f"rb{nm}")
                        nc.gpsimd.partition_broadcast(rb, rinv, channels=128)
                        nc.vector.tensor_mul(
                            out=yt[:, h, SB * b : SB * (b + 1)],
                            in0=yacc,
                            in1=rb,
                        )

            # ============ Phase O: out-projection partials ============
                for b in range(NBLK):
                    for dg in range(NDT // 4):
                        ot4 = sc.tile(
                            [128, 4, NB], bf16, tag="ot4", bufs=3, name=f"ot{dg}_{b}"
                        )
                        for dl in range(4):
                            dm = 4 * dg + dl
                            oacc = psA.tile(
                                [128, NB], f32, tag="y", name=f"oacc{dm}_{b}"
                            )
                            for y in range(H):
                                nc.tensor.matmul(
                                    oacc,
                                    lhsT=wo_sb[:, y, 128 * dm : 128 * (dm + 1)],
                                    rhs=yt[:, y, SB * b : SB * (b + 1)],
                                    start=(y == 0),
                                    stop=(y == H - 1),
                                )
                            nc.vector.tensor_copy(out=ot4[:, dl, :], in_=oacc)
                        nc.sync.dma_start(
                            out=obounce[
                                512 * dg : 512 * (dg + 1), SB * b : SB * (b + 1)
                            ].rearrange("(c p) w -> p c w", p=128),
                            in_=ot4,
                        )

            nc.gpsimd.collective_compute(
                "ReduceScatter",
                mybir.AluOpType.add,
                replica_groups=[list(range(NCORE))],
                ins=[obounce.opt()],
                outs=[oslice.opt()],
            )
            nc.gpsimd.dma_start(outp, oslice[:])

    nc.compile()
    return nc


_tables = None


def _rope_tables():
    """Input-independent cos/sin tables [64, S] in bf16; cached per process."""
    global _tables
    if _tables is None:
        pos = np.arange(S, dtype=np.float32)
        invf = 1.0 / (10000.0 ** (np.arange(0, HD, 2, dtype=np.float32) / HD))
        fr = pos[:, None] * invf[None, :]
        _tables = (
            np.cos(fr).T.astype(ml_dtypes.bfloat16),
            np.sin(fr).T.astype(ml_dtypes.bfloat16),
        )
    return _tables


def host_prep(x, Wq, Wk, Wv, Wo):
    x2 = np.asarray(x, dtype=np.float32).reshape(S, D)
    xT = x2.T.astype(ml_dtypes.bfloat16)  # F-order: cheap single-pass convert
    WqT = np.asarray(Wq, np.float32).T.astype(ml_dtypes.bfloat16)
    WkT = np.asarray(Wk, np.float32).T.astype(ml_dtypes.bfloat16)
    WvT = np.asarray(Wv, np.float32).T.astype(ml_dtypes.bfloat16)
    WoT = np.asarray(Wo, np.float32).T
    C64, S64 = _rope_tables()

    in_maps = []
    for c in range(NCORE):
        sl = slice(SB * c, SB * (c + 1))
        gin = np.empty((GROWS, SB), ml_dtypes.bfloat16)
        gin[:D] = xT[:, sl]
        gin[D : D + 64] = C64[:, sl]
        gin[D + 64 :] = S64[:, sl]
        kvh = c // 2
        Whalf = WkT if c % 2 == 0 else WvT
        in_maps.append(
            dict(
                gin=gin,
                wqd=np.ascontiguousarray(WqT[:, OW * c : OW * (c + 1)]),
                wkvh=np.ascontiguousarray(Whalf[:, HD * kvh : HD * (kvh + 1)]),
                wod=np.ascontiguousarray(
                    WoT[OW * c : OW * (c + 1), :].astype(ml_dtypes.bfloat16)
                ),
            )
        )
    return in_maps


def host_post(results):
    out = np.empty((S, D), np.float32)
    for c in range(NCORE):
        out[:, OW * c : OW * (c + 1)] = results[c]["outp"].T
    return out.reshape(1, S, D)


_cached_nc = None


def get_nc():
    global _cached_nc
    if _cached_nc is None:
        _cached_nc = build_program()
    return _cached_nc


def kernel(**inputs):
    nc = get_nc()
    in_maps = host_prep(
        inputs["x"], inputs["Wq"], inputs["Wk"], inputs["Wv"], inputs["Wo"]
    )
    res = bass_utils.run_bass_kernel_spmd(nc, in_maps, core_ids=list(range(8)))
    return host_post(res.results)
